# revision 10
# baseline (speedup 1.0000x reference)
"""EnhancedBVHRouter forward on 8 TRN2 NeuronCores (Bass/Tile).

Strategy (pure data parallel, feature-major on device):
 - Host: shard tokens 8-way; transpose + cast x to bf16 (2048 x 4096 per core);
   fold all router parameters (LayerNorm affine, f2w compositions, geo term)
   into a small set of stationary matrices so each 512-token chunk is a
   chain of matmuls (features on partitions, tokens on the free dim).
 - Device per 512-token chunk:
     h1 = gelu(pw1.T @ xT + pb1)                 [bf16 matmuls, 64 MMs]
     h0 = pw2.T @ h1 + pb2                       [f32r]
     LayerNorm stats via ones-matmuls; hhat scaling via rank-1 matmul trick
     3 BVH levels: pre/pos/logit matmuls with host-folded stationaries,
       softmax over 4 children via exp + blockdiag-ones matmuls
     head: gelu(C3a.T @ u3 + Hp.T @ prows + bh); logits = hw2.T @ g + hb2
     expert softmax over 64 via exp + ones-matmuls; probsT written out
 - Host: gather probsT shards, transpose, argmax -> int32 ids.

The -AT*|pos|^2 term in the level logits is constant across the 4 children,
so it is dropped (softmax shift invariance); |c|^2 folds into the bias.
"""
import os
import numpy as np
import ml_dtypes

import concourse.bass as bass
import concourse.tile as tile
from concourse import bacc, mybir
from concourse.bass_utils import run_bass_kernel_spmd

N_CORES = 8
B, D = 32768, 2048
TOK = B // N_CORES            # tokens per core
CHUNK = 512                   # tokens per pipeline chunk
NCH = TOK // CHUNK
AT = 0.5 / (2.0 + 1e-8)       # coefficient of d_sq inside the level logits

dt = mybir.dt
AF = mybir.ActivationFunctionType
ALU = mybir.AluOpType

# BIAS matrix column layout (128 x 18 fp32)
BCOL_PB1 = 0      # 4 cols: pb1 per output tile
BCOL_PB2 = 4      # 2 cols
BCOL_A1 = 6       # pre1 bias
BCOL_P1 = 7       # pos1 bias (rows 0:3)
BCOL_C1A = 8      # pre2 bias
BCOL_C1W = 9      # pos2 bias (rows 0:3)
BCOL_B1 = 10      # L1 bias (rows 0:4)
BCOL_C2A = 11     # pre3 bias
BCOL_C2W = 12     # pos3 bias (rows 0:3)
BCOL_B2 = 13      # L2 bias (rows 0:4)
BCOL_B3 = 14      # L3 bias (rows 0:4)
BCOL_BH = 15      # 2 cols: head bias
BCOL_HB2 = 17     # expert logits bias (rows 0:64)
BCOL_EPS = 18     # row 0: LN epsilon
NBCOL = 19

_CACHE = {}


def _fold(d):
    """Host-side folding, fp64 -> fp32 arrays for the device."""
    P = {k: np.asarray(v, np.float64) for k, v in d.items()}
    lng, lnb = P["lng"], P["lnb"]
    f = {}
    A1 = lng[:, None] * P["l1_f1w"]          # (256,128)
    P1 = lng[:, None] * P["l1_w3"]           # (256,3)
    f["lnA"] = np.concatenate([A1, P1], axis=1)            # (256,131)
    f["CROW"] = np.concatenate([A1.sum(0), P1.sum(0)])[None, :]  # (1,131)
    a1 = P["l1_f1b"] + lnb @ P["l1_f1w"]
    p1b = P["l1_b3"] + lnb @ P["l1_w3"]

    def lvl_fold(l):
        rw = P[f"l{l}_rw"]
        Crw = P[f"l{l}_f2w"] @ rw[:128]                     # (128,4)
        Wp = rw[128:131] + 2.0 * AT * P[f"l{l}_c"].T        # (3,4)
        bL = P[f"l{l}_rb"] + P[f"l{l}_f2b"] @ rw[:128] \
            - AT * np.sum(P[f"l{l}_c"] ** 2, axis=-1)
        return Crw, Wp, bL

    C1rw, Wp1, b1 = lvl_fold(1)
    C2rw, Wp2, b2 = lvl_fold(2)
    C3rw, Wp3, b3 = lvl_fold(3)
    C1a = P["l1_f2w"] @ P["l2_f1w"]
    c1a = P["l2_f1b"] + P["l1_f2b"] @ P["l2_f1w"]
    C1w3 = P["l1_f2w"] @ P["l2_w3"]
    c1w = P["l2_b3"] + P["l1_f2b"] @ P["l2_w3"]
    C2a = P["l2_f2w"] @ P["l3_f1w"]
    c2a = P["l3_f1b"] + P["l2_f2b"] @ P["l3_f1w"]
    C2w3 = P["l2_f2w"] @ P["l3_w3"]
    c2w = P["l3_b3"] + P["l2_f2b"] @ P["l3_w3"]
    C3a = P["l3_f2w"] @ P["hw1"][:128]                      # (128,256)
    bh = P["hb1"] + P["l3_f2b"] @ P["hw1"][:128]

    f["C1"] = np.concatenate([C1a, C1rw, C1w3], axis=1)     # (128,135)
    f["C2"] = np.concatenate([C2a, C2rw, C2w3], axis=1)     # (128,135)
    f["C3"] = np.concatenate([C3a, C3rw], axis=1)           # (128,260)
    f["Wp"] = np.concatenate([Wp1, Wp2, Wp3], axis=1)       # (3,12)
    hp68 = np.zeros((68, 256))
    for l in range(3):
        hp68[32 * l:32 * l + 4] = P["hw1"][128 + 4 * l:132 + 4 * l]
    f["Hp"] = hp68                                          # (68,256)
    f["hw2"] = P["hw2"]                                     # (256,64)
    f["pw2"] = P["pw2"]                                     # (512,256)

    bd = np.zeros((68, 3))
    for l in range(3):
        bd[32 * l:32 * l + 4, l] = 1.0
    f["BD"] = bd
    f["BDT"] = bd.T.copy()

    bias = np.zeros((128, NBCOL))
    for mo in range(4):
        bias[:, BCOL_PB1 + mo] = P["pb1"][mo * 128:(mo + 1) * 128]
    for mo in range(2):
        bias[:, BCOL_PB2 + mo] = P["pb2"][mo * 128:(mo + 1) * 128]
    bias[:, BCOL_A1] = a1
    bias[0:3, BCOL_P1] = p1b
    bias[:, BCOL_C1A] = c1a
    bias[0:3, BCOL_C1W] = c1w
    bias[0:4, BCOL_B1] = b1
    bias[:, BCOL_C2A] = c2a
    bias[0:3, BCOL_C2W] = c2w
    bias[0:4, BCOL_B2] = b2
    bias[0:4, BCOL_B3] = b3
    for mo in range(2):
        bias[:, BCOL_BH + mo] = bh[mo * 128:(mo + 1) * 128]
    bias[0:64, BCOL_HB2] = P["hb2"][0:64]
    bias[0, BCOL_EPS] = 1e-5
    f["BIAS"] = bias
    return {k: np.ascontiguousarray(v, np.float32) for k, v in f.items()}


def _build():
    nc = bacc.Bacc("TRN2", target_bir_lowering=False, debug=False,
                   num_devices=N_CORES)
    xT = nc.dram_tensor("xT", [D, TOK], dt.bfloat16, kind="ExternalInput").ap()
    pw1 = nc.dram_tensor("pw1", [D, 512], dt.bfloat16, kind="ExternalInput").ap()
    pw2 = nc.dram_tensor("pw2", [512, 256], dt.float32r, kind="ExternalInput").ap()
    lnA = nc.dram_tensor("lnA", [256, 131], dt.float32r, kind="ExternalInput").ap()
    CROW = nc.dram_tensor("CROW", [1, 131], dt.float32r, kind="ExternalInput").ap()
    C1 = nc.dram_tensor("C1", [128, 135], dt.float32r, kind="ExternalInput").ap()
    C2 = nc.dram_tensor("C2", [128, 135], dt.float32r, kind="ExternalInput").ap()
    C3 = nc.dram_tensor("C3", [128, 260], dt.float32r, kind="ExternalInput").ap()
    Wp = nc.dram_tensor("Wp", [3, 12], dt.float32r, kind="ExternalInput").ap()
    Hp = nc.dram_tensor("Hp", [68, 256], dt.float32r, kind="ExternalInput").ap()
    hw2 = nc.dram_tensor("hw2", [256, 64], dt.float32r, kind="ExternalInput").ap()
    BD = nc.dram_tensor("BD", [68, 3], dt.float32r, kind="ExternalInput").ap()
    BDT = nc.dram_tensor("BDT", [3, 68], dt.float32r, kind="ExternalInput").ap()
    BIAS = nc.dram_tensor("BIAS", [128, NBCOL], dt.float32, kind="ExternalInput").ap()
    ONES = nc.dram_tensor("ONES", [128, 128], dt.float32r, kind="ExternalInput").ap()
    ONES32 = nc.dram_tensor("ONES32", [64, 64], dt.float32, kind="ExternalInput").ap()
    probsT = nc.dram_tensor("probsT", [64, TOK], dt.float32, kind="ExternalOutput").ap()
    dbg = os.environ.get("KDEBUG", "0") == "1"
    if dbg:
        dbg_h0 = nc.dram_tensor("dbg_h0", [128, CHUNK], dt.float32, kind="ExternalOutput").ap()
        dbg_aa = nc.dram_tensor("dbg_aa", [1, CHUNK], dt.float32, kind="ExternalOutput").ap()
        dbg_u1 = nc.dram_tensor("dbg_u1", [128, CHUNK], dt.float32, kind="ExternalOutput").ap()
        dbg_er = nc.dram_tensor("dbg_er", [68, CHUNK], dt.float32, kind="ExternalOutput").ap()
        dbg_g = nc.dram_tensor("dbg_g", [128, CHUNK], dt.float32, kind="ExternalOutput").ap()
        dbg_E = nc.dram_tensor("dbg_E", [64, CHUNK], dt.float32, kind="ExternalOutput").ap()

    with tile.TileContext(nc) as tc, nc.allow_low_precision(reason="f32r tiles are 4-byte"):
        with tc.tile_pool(name="weights", bufs=1) as wp, \
             tc.tile_pool(name="x", bufs=2) as xp, \
             tc.tile_pool(name="inter", bufs=2) as ip, \
             tc.tile_pool(name="psB", bufs=3, space="PSUM") as psB, \
             tc.tile_pool(name="psM", bufs=3, space="PSUM") as psM, \
             tc.tile_pool(name="psC", bufs=2, space="PSUM") as psC:

            # ---- persistent weights ----
            pw1_t = []
            for k in range(16):
                t = wp.tile([128, 512], dt.bfloat16, name=f"pw1_{k}")
                nc.sync.dma_start(t[:], pw1[k * 128:(k + 1) * 128, :])
                pw1_t.append(t)
            pw2_t = []
            for k in range(4):
                t = wp.tile([128, 256], dt.float32r, name=f"pw2_{k}")
                nc.sync.dma_start(t[:], pw2[k * 128:(k + 1) * 128, :])
                pw2_t.append(t)
            lnA_t = []
            for k in range(2):
                t = wp.tile([128, 131], dt.float32r, name=f"lnA_{k}")
                nc.sync.dma_start(t[:], lnA[k * 128:(k + 1) * 128, :])
                lnA_t.append(t)
            crow_t = wp.tile([1, 131], dt.float32r, name="crow")
            nc.sync.dma_start(crow_t[:], CROW[:, :])
            c1_t = wp.tile([128, 135], dt.float32r, name="c1")
            nc.sync.dma_start(c1_t[:], C1[:, :])
            c2_t = wp.tile([128, 135], dt.float32r, name="c2")
            nc.sync.dma_start(c2_t[:], C2[:, :])
            c3_t = wp.tile([128, 260], dt.float32r, name="c3")
            nc.sync.dma_start(c3_t[:], C3[:, :])
            wp_t = wp.tile([3, 12], dt.float32r, name="wp_t")
            nc.sync.dma_start(wp_t[:], Wp[:, :])
            hp_t = wp.tile([68, 256], dt.float32r, name="hp_t")
            nc.sync.dma_start(hp_t[:], Hp[:, :])
            hw2_t = []
            for k in range(2):
                t = wp.tile([128, 64], dt.float32r, name=f"hw2_{k}")
                nc.sync.dma_start(t[:], hw2[k * 128:(k + 1) * 128, :])
                hw2_t.append(t)
            bd_t = wp.tile([68, 3], dt.float32r, name="bd_t")
            nc.sync.dma_start(bd_t[:], BD[:, :])
            bdt_t = wp.tile([3, 68], dt.float32r, name="bdt_t")
            nc.sync.dma_start(bdt_t[:], BDT[:, :])
            bias_t = wp.tile([128, NBCOL], dt.float32, name="bias_t")
            nc.sync.dma_start(bias_t[:], BIAS[:, :])
            ones_t = wp.tile([128, 128], dt.float32r, name="ones_t")
            nc.sync.dma_start(ones_t[:], ONES[:, :])
            ones32_t = wp.tile([64, 64], dt.float32, name="ones32_t")
            nc.sync.dma_start(ones32_t[:], ONES32[:, :])

            def bcol(c, rows=128):
                return bias_t[0:rows, c:c + 1]

            # ---- per-chunk pipeline ----
            for ch in range(NCH):
                t0 = ch * CHUNK
                xt = []
                for k in range(16):
                    t = xp.tile([128, CHUNK], dt.bfloat16, tag=f"xt{k}", name=f"xt{k}_{ch}")
                    nc.sync.dma_start(t[:], xT[k * 128:(k + 1) * 128, t0:t0 + CHUNK])
                    xt.append(t)

                # input_proj layer 1: h1 = gelu(pw1.T @ xT + pb1)
                h1 = []
                for mo in range(4):
                    ps = psB.tile([128, CHUNK], dt.float32, tag="psb", name=f"ps_h1_{ch}_{mo}")
                    for k in range(16):
                        nc.tensor.matmul(ps[:], pw1_t[k][:, mo * 128:(mo + 1) * 128],
                                         xt[k][:], start=(k == 0), stop=(k == 15))
                    t = ip.tile([128, CHUNK], dt.float32r, tag=f"h1_{mo}", name=f"h1_{ch}_{mo}")
                    nc.scalar.activation(t[:], ps[:], AF.Gelu, bias=bcol(BCOL_PB1 + mo))
                    h1.append(t)

                # input_proj layer 2: h0 = pw2.T @ h1 + pb2
                h0 = []
                for mo in range(2):
                    ps = psB.tile([128, CHUNK], dt.float32, tag="psb", name=f"ps_h0_{ch}_{mo}")
                    for k in range(4):
                        nc.tensor.matmul(ps[:], pw2_t[k][:, mo * 128:(mo + 1) * 128],
                                         h1[k][:], start=(k == 0), stop=(k == 3))
                    t = ip.tile([128, CHUNK], dt.float32r, tag=f"h0_{mo}", name=f"h0_{ch}_{mo}")
                    nc.vector.tensor_scalar_add(t[:], ps[:], bcol(BCOL_PB2 + mo))
                    h0.append(t)

                # LayerNorm stats: sum and sum-of-squares over 256 features
                hsq = []
                for mo in range(2):
                    t = ip.tile([128, CHUNK], dt.float32r, tag=f"hsq_{mo}", name=f"hsq_{ch}_{mo}")
                    nc.vector.tensor_mul(t[:], h0[mo][:], h0[mo][:])
                    hsq.append(t)
                st0 = psM.tile([1, CHUNK], dt.float32, tag="psm", name=f"ps_st0_{ch}")
                for mo in range(2):
                    nc.tensor.matmul(st0[:], ones_t[:, 0:1], h0[mo][:],
                                     start=(mo == 0), stop=(mo == 1))
                st1 = psM.tile([1, CHUNK], dt.float32, tag="psm", name=f"ps_st1_{ch}")
                for mo in range(2):
                    nc.tensor.matmul(st1[:], ones_t[:, 0:1], hsq[mo][:],
                                     start=(mo == 0), stop=(mo == 1))
                # mu = st0/256 ; var = (st1 - st0^2/256)/256 ; aa = rstd ; bb = -mu*rstd
                tm = ip.tile([1, CHUNK], dt.float32, tag="scr", name=f"tm_{ch}")
                nc.scalar.activation(tm[:], st0[:], AF.Square)
                vv = ip.tile([1, CHUNK], dt.float32, tag="scr", name=f"vv_{ch}")
                nc.vector.scalar_tensor_tensor(vv[:], tm[:], -1.0 / 256.0, st1[:],
                                               op0=ALU.mult, op1=ALU.add)
                sd = ip.tile([1, CHUNK], dt.float32, tag="scr", name=f"sd_{ch}")
                nc.scalar.activation(sd[:], vv[:], AF.Sqrt, scale=1.0 / 256.0,
                                     bias=bias_t[0:1, BCOL_EPS:BCOL_EPS + 1])
                aa = ip.tile([1, CHUNK], dt.float32r, tag="aa", name=f"aa_{ch}")
                nc.vector.reciprocal(aa[:], sd[:])
                bb = ip.tile([1, CHUNK], dt.float32r, tag="bb", name=f"bb_{ch}")
                nc.vector.scalar_tensor_tensor(bb[:], st0[:], -1.0 / 256.0, aa[:],
                                               op0=ALU.mult, op1=ALU.mult)
                abc = psC.tile([128, CHUNK], dt.float32, tag="psc", name=f"ps_ab_{ch}")
                nc.tensor.matmul(abc[:], ones_t[0:1, 0:128], aa[:], start=True, stop=True)
                h0s = []
                for mo in range(2):
                    t = ip.tile([128, CHUNK], dt.float32r, tag=f"h0s_{mo}", name=f"h0s_{ch}_{mo}")
                    nc.vector.tensor_mul(t[:], h0[mo][:], abc[:])
                    h0s.append(t)

                # level 1 pre/pos:  hhat @ [A1|P1]  (+ rank-1 bb term)
                ps = psB.tile([128, CHUNK], dt.float32, tag="psb", name=f"ps_pre1_{ch}")
                for k in range(2):
                    nc.tensor.matmul(ps[:], lnA_t[k][:, 0:128], h0s[k][:],
                                     start=(k == 0), stop=False)
                nc.tensor.matmul(ps[:], crow_t[0:1, 0:128], bb[:], start=False, stop=True)
                u1 = ip.tile([128, CHUNK], dt.float32r, tag="u1", name=f"u1_{ch}")
                nc.scalar.activation(u1[:], ps[:], AF.Gelu, bias=bcol(BCOL_A1))

                pp = psM.tile([3, CHUNK], dt.float32, tag="psm", name=f"ps_pos1_{ch}")
                for k in range(2):
                    nc.tensor.matmul(pp[:], lnA_t[k][:, 128:131], h0s[k][:],
                                     start=(k == 0), stop=False)
                nc.tensor.matmul(pp[:], crow_t[0:1, 128:131], bb[:], start=False, stop=True)
                pos1 = ip.tile([3, CHUNK], dt.float32r, tag="pos", name=f"pos1_{ch}")
                nc.scalar.activation(pos1[:], pp[:], AF.Identity, bias=bcol(BCOL_P1, 3))

                erows = ip.tile([68, CHUNK], dt.float32r, tag="erows", name=f"erows_{ch}")

                def level_bl(lv, ct, u, pos, bcol_L, bcol_w3, nxt_pos_name):
                    """L = Crw.T@u + Wp.T@pos -> exp into erows rows; pos_next = Cw3.T@u."""
                    off = 128 if lv < 3 else 256
                    pl = psM.tile([4, CHUNK], dt.float32, tag="psm", name=f"ps_bl{lv}_{ch}")
                    nc.tensor.matmul(pl[:], ct[:, off:off + 4], u[:], start=True, stop=False)
                    nc.tensor.matmul(pl[:], wp_t[:, 4 * (lv - 1):4 * lv], pos[:],
                                     start=False, stop=True)
                    nc.scalar.activation(erows[32 * (lv - 1):32 * (lv - 1) + 4, :],
                                         pl[:], AF.Exp, bias=bcol(bcol_L, 4))
                    if bcol_w3 is not None:
                        pw = psM.tile([3, CHUNK], dt.float32, tag="psm", name=f"ps_w3{lv}_{ch}")
                        nc.tensor.matmul(pw[:], ct[:, off + 4:off + 7], u[:],
                                         start=True, stop=True)
                        pn = ip.tile([3, CHUNK], dt.float32r, tag="pos", name=nxt_pos_name)
                        nc.scalar.activation(pn[:], pw[:], AF.Identity,
                                             bias=bcol(bcol_w3, 3))
                        return pn
                    return None

                # level 1 -> pre2, L1, pos2
                ps = psB.tile([128, CHUNK], dt.float32, tag="psb", name=f"ps_pre2_{ch}")
                nc.tensor.matmul(ps[:], c1_t[:, 0:128], u1[:], start=True, stop=True)
                u2 = ip.tile([128, CHUNK], dt.float32r, tag="u2", name=f"u2_{ch}")
                nc.scalar.activation(u2[:], ps[:], AF.Gelu, bias=bcol(BCOL_C1A))
                pos2 = level_bl(1, c1_t, u1, pos1, BCOL_B1, BCOL_C1W, f"pos2_{ch}")

                # level 2 -> pre3, L2, pos3
                ps = psB.tile([128, CHUNK], dt.float32, tag="psb", name=f"ps_pre3_{ch}")
                nc.tensor.matmul(ps[:], c2_t[:, 0:128], u2[:], start=True, stop=True)
                u3 = ip.tile([128, CHUNK], dt.float32r, tag="u3", name=f"u3_{ch}")
                nc.scalar.activation(u3[:], ps[:], AF.Gelu, bias=bcol(BCOL_C2A))
                pos3 = level_bl(2, c2_t, u2, pos2, BCOL_B2, BCOL_C2W, f"pos3_{ch}")

                # level 3 -> L3
                level_bl(3, c3_t, u3, pos3, BCOL_B3, None, None)

                # level softmax: S = BD.T @ erows ; prows = erows * bcast(1/S)
                sl = psM.tile([3, CHUNK], dt.float32, tag="psm", name=f"ps_S_{ch}")
                nc.tensor.matmul(sl[:], bd_t[:], erows[:], start=True, stop=True)
                lrec = ip.tile([3, CHUNK], dt.float32r, tag="lrec", name=f"lrec_{ch}")
                nc.vector.reciprocal(lrec[:], sl[:])
                lbc = psC.tile([68, CHUNK], dt.float32, tag="psc", name=f"ps_lbc_{ch}")
                nc.tensor.matmul(lbc[:], bdt_t[:], lrec[:], start=True, stop=True)
                prows = ip.tile([68, CHUNK], dt.float32r, tag="prows", name=f"prows_{ch}")
                nc.vector.tensor_mul(prows[:], erows[:], lbc[:])

                # head: g = gelu(C3a.T @ u3 + Hp.T @ prows + bh)
                g = []
                for mo in range(2):
                    ps = psB.tile([128, CHUNK], dt.float32, tag="psb", name=f"ps_hd_{ch}_{mo}")
                    nc.tensor.matmul(ps[:], c3_t[:, mo * 128:(mo + 1) * 128], u3[:],
                                     start=True, stop=False)
                    nc.tensor.matmul(ps[:], hp_t[:, mo * 128:(mo + 1) * 128], prows[:],
                                     start=False, stop=True)
                    t = ip.tile([128, CHUNK], dt.float32r, tag=f"g_{mo}", name=f"g_{ch}_{mo}")
                    nc.scalar.activation(t[:], ps[:], AF.Gelu, bias=bcol(BCOL_BH + mo))
                    g.append(t)

                # expert logits + softmax
                pe = psM.tile([64, CHUNK], dt.float32, tag="psm", name=f"ps_E_{ch}")
                for k in range(2):
                    nc.tensor.matmul(pe[:], hw2_t[k][:], g[k][:],
                                     start=(k == 0), stop=(k == 1))
                eexp = ip.tile([64, CHUNK], dt.float32, tag="eexp", name=f"eexp_{ch}")
                nc.scalar.activation(eexp[:], pe[:], AF.Exp, bias=bcol(BCOL_HB2, 64))
                se = psM.tile([1, CHUNK], dt.float32, tag="psm", name=f"ps_se_{ch}")
                nc.tensor.matmul(se[:], ones32_t[0:64, 0:1], eexp[:], start=True, stop=True)
                erec = ip.tile([1, CHUNK], dt.float32, tag="erec", name=f"erec_{ch}")
                nc.vector.reciprocal(erec[:], se[:])
                ebc = psC.tile([64, CHUNK], dt.float32, tag="psc", name=f"ps_ebc_{ch}")
                nc.tensor.matmul(ebc[:], ones32_t[0:1, 0:64], erec[:], start=True, stop=True)
                pout = ip.tile([64, CHUNK], dt.float32, tag="pout", name=f"pout_{ch}")
                nc.vector.tensor_mul(pout[:], eexp[:], ebc[:])
                nc.sync.dma_start(probsT[:, t0:t0 + CHUNK], pout[:])
                if dbg and ch == 0:
                    nc.sync.dma_start(dbg_h0[:, :], h0[0][:].bitcast(dt.float32))
                    nc.sync.dma_start(dbg_aa[:, :], aa[:].bitcast(dt.float32))
                    nc.sync.dma_start(dbg_u1[:, :], u1[:].bitcast(dt.float32))
                    nc.sync.dma_start(dbg_er[:, :], erows[:].bitcast(dt.float32))
                    nc.sync.dma_start(dbg_g[:, :], g[0][:].bitcast(dt.float32))
                    eL = ip.tile([64, CHUNK], dt.float32, tag="eL", name="eL_dbg")
                    nc.scalar.activation(eL[:], pe[:], AF.Identity)
                    nc.sync.dma_start(dbg_E[:, :], eL[:])

    nc.compile()
    return nc


def _prepare_in_maps(inputs):
    f = _fold(inputs)
    x = np.asarray(inputs["x"], np.float32)
    pw1_bf = np.ascontiguousarray(np.asarray(inputs["pw1"], np.float32)
                                  .astype(ml_dtypes.bfloat16))
    xbf = x.astype(ml_dtypes.bfloat16)
    shared = {
        "pw1": pw1_bf, "pw2": f["pw2"], "lnA": f["lnA"], "CROW": f["CROW"],
        "C1": f["C1"], "C2": f["C2"], "C3": f["C3"], "Wp": f["Wp"],
        "Hp": f["Hp"], "hw2": f["hw2"], "BD": f["BD"], "BDT": f["BDT"],
        "BIAS": f["BIAS"], "ONES": np.ones((128, 128), np.float32),
        "ONES32": np.ones((64, 64), np.float32),
    }
    in_maps = []
    for c in range(N_CORES):
        xTc = np.ascontiguousarray(xbf[c * TOK:(c + 1) * TOK].T)
        in_maps.append({"xT": xTc, **shared})
    return in_maps


def kernel(**inputs):
    if "nc" not in _CACHE:
        _CACHE["nc"] = _build()
    nc = _CACHE["nc"]
    in_maps = _prepare_in_maps(inputs)
    res = run_bass_kernel_spmd(nc, in_maps, core_ids=list(range(N_CORES)))
    probsT = np.concatenate([r["probsT"] for r in res.results], axis=1)
    probs = np.ascontiguousarray(probsT.T)
    ids = np.argmax(probs, axis=-1).astype(np.int32)
    return probs, ids


# revision 11
# speedup vs baseline: 1.3872x; 1.3872x over previous
"""EnhancedBVHRouter forward on 8 TRN2 NeuronCores (Bass/Tile).

Strategy (pure data parallel, feature-major on device):
 - Host: shard tokens 8-way; transpose + cast x to bf16 (2048 x 4096 per core);
   fold all router parameters (LayerNorm affine, f2w compositions, geo term)
   into a small set of stationary matrices so each 512-token chunk is a
   chain of matmuls (features on partitions, tokens on the free dim).
 - Device per 512-token chunk:
     h1 = gelu(pw1.T @ xT + pb1)                 [bf16 matmuls, 64 MMs]
     h0 = pw2.T @ h1 + pb2                       [f32r]
     LayerNorm stats via ones-matmuls; hhat scaling via rank-1 matmul trick
     3 BVH levels: pre/pos/logit matmuls with host-folded stationaries,
       softmax over 4 children via exp + blockdiag-ones matmuls
     head: gelu(C3a.T @ u3 + Hp.T @ prows + bh); logits = hw2.T @ g + hb2
     expert softmax over 64 via exp + ones-matmuls; probsT written out
 - Host: gather probsT shards, transpose, argmax -> int32 ids.

The -AT*|pos|^2 term in the level logits is constant across the 4 children,
so it is dropped (softmax shift invariance); |c|^2 folds into the bias.
"""
import os
import numpy as np
import ml_dtypes

import concourse.bass as bass
import concourse.tile as tile
from concourse import bacc, mybir
from concourse.bass_utils import run_bass_kernel_spmd

N_CORES = 8
B, D = 32768, 2048
TOK = B // N_CORES            # tokens per core
CHUNK = 512                   # tokens per pipeline chunk
NCH = TOK // CHUNK
AT = 0.5 / (2.0 + 1e-8)       # coefficient of d_sq inside the level logits

dt = mybir.dt
AF = mybir.ActivationFunctionType
ALU = mybir.AluOpType

# BIAS matrix column layout (128 x 18 fp32)
BCOL_PB1 = 0      # 4 cols: pb1 per output tile
BCOL_PB2 = 4      # 2 cols
BCOL_A1 = 6       # pre1 bias
BCOL_P1 = 7       # pos1 bias (rows 0:3)
BCOL_C1A = 8      # pre2 bias
BCOL_C1W = 9      # pos2 bias (rows 0:3)
BCOL_B1 = 10      # L1 bias (rows 0:4)
BCOL_C2A = 11     # pre3 bias
BCOL_C2W = 12     # pos3 bias (rows 0:3)
BCOL_B2 = 13      # L2 bias (rows 0:4)
BCOL_B3 = 14      # L3 bias (rows 0:4)
BCOL_BH = 15      # 2 cols: head bias
BCOL_HB2 = 17     # expert logits bias (rows 0:64)
BCOL_EPS = 18     # row 0: LN epsilon
NBCOL = 19

_CACHE = {}


def _fold(d):
    """Host-side folding, fp64 -> fp32 arrays for the device."""
    P = {k: np.asarray(v, np.float64) for k, v in d.items()}
    lng, lnb = P["lng"], P["lnb"]
    f = {}
    A1 = lng[:, None] * P["l1_f1w"]          # (256,128)
    P1 = lng[:, None] * P["l1_w3"]           # (256,3)
    f["lnA"] = np.concatenate([A1, P1], axis=1)            # (256,131)
    f["CROW"] = np.concatenate([A1.sum(0), P1.sum(0)])[None, :]  # (1,131)
    a1 = P["l1_f1b"] + lnb @ P["l1_f1w"]
    p1b = P["l1_b3"] + lnb @ P["l1_w3"]

    def lvl_fold(l):
        rw = P[f"l{l}_rw"]
        Crw = P[f"l{l}_f2w"] @ rw[:128]                     # (128,4)
        Wp = rw[128:131] + 2.0 * AT * P[f"l{l}_c"].T        # (3,4)
        bL = P[f"l{l}_rb"] + P[f"l{l}_f2b"] @ rw[:128] \
            - AT * np.sum(P[f"l{l}_c"] ** 2, axis=-1)
        return Crw, Wp, bL

    C1rw, Wp1, b1 = lvl_fold(1)
    C2rw, Wp2, b2 = lvl_fold(2)
    C3rw, Wp3, b3 = lvl_fold(3)
    C1a = P["l1_f2w"] @ P["l2_f1w"]
    c1a = P["l2_f1b"] + P["l1_f2b"] @ P["l2_f1w"]
    C1w3 = P["l1_f2w"] @ P["l2_w3"]
    c1w = P["l2_b3"] + P["l1_f2b"] @ P["l2_w3"]
    C2a = P["l2_f2w"] @ P["l3_f1w"]
    c2a = P["l3_f1b"] + P["l2_f2b"] @ P["l3_f1w"]
    C2w3 = P["l2_f2w"] @ P["l3_w3"]
    c2w = P["l3_b3"] + P["l2_f2b"] @ P["l3_w3"]
    C3a = P["l3_f2w"] @ P["hw1"][:128]                      # (128,256)
    bh = P["hb1"] + P["l3_f2b"] @ P["hw1"][:128]

    f["C1"] = np.concatenate([C1a, C1rw, C1w3], axis=1)     # (128,135)
    f["C2"] = np.concatenate([C2a, C2rw, C2w3], axis=1)     # (128,135)
    f["C3"] = np.concatenate([C3a, C3rw], axis=1)           # (128,260)
    f["Wp"] = np.concatenate([Wp1, Wp2, Wp3], axis=1)       # (3,12)
    hp68 = np.zeros((68, 256))
    for l in range(3):
        hp68[32 * l:32 * l + 4] = P["hw1"][128 + 4 * l:132 + 4 * l]
    f["Hp"] = hp68                                          # (68,256)
    f["hw2"] = P["hw2"]                                     # (256,64)
    f["pw2"] = P["pw2"]                                     # (512,256)

    bd = np.zeros((68, 3))
    for l in range(3):
        bd[32 * l:32 * l + 4, l] = 1.0
    f["BD"] = bd
    f["BDT"] = bd.T.copy()

    bias = np.zeros((128, NBCOL))
    for mo in range(4):
        bias[:, BCOL_PB1 + mo] = P["pb1"][mo * 128:(mo + 1) * 128]
    for mo in range(2):
        bias[:, BCOL_PB2 + mo] = P["pb2"][mo * 128:(mo + 1) * 128]
    bias[:, BCOL_A1] = a1
    bias[0:3, BCOL_P1] = p1b
    bias[:, BCOL_C1A] = c1a
    bias[0:3, BCOL_C1W] = c1w
    bias[0:4, BCOL_B1] = b1
    bias[:, BCOL_C2A] = c2a
    bias[0:3, BCOL_C2W] = c2w
    bias[0:4, BCOL_B2] = b2
    bias[0:4, BCOL_B3] = b3
    for mo in range(2):
        bias[:, BCOL_BH + mo] = bh[mo * 128:(mo + 1) * 128]
    bias[0:64, BCOL_HB2] = P["hb2"][0:64]
    bias[0, BCOL_EPS] = 1e-5
    f["BIAS"] = bias
    return {k: np.ascontiguousarray(v, np.float32) for k, v in f.items()}


def _build():
    nc = bacc.Bacc("TRN2", target_bir_lowering=False, debug=False,
                   num_devices=N_CORES)
    xT = nc.dram_tensor("xT", [D, TOK], dt.bfloat16, kind="ExternalInput").ap()
    pw1 = nc.dram_tensor("pw1", [D, 512], dt.bfloat16, kind="ExternalInput").ap()
    pw2 = nc.dram_tensor("pw2", [512, 256], dt.float32r, kind="ExternalInput").ap()
    lnA = nc.dram_tensor("lnA", [256, 131], dt.float32r, kind="ExternalInput").ap()
    CROW = nc.dram_tensor("CROW", [1, 131], dt.float32r, kind="ExternalInput").ap()
    C1 = nc.dram_tensor("C1", [128, 135], dt.float32r, kind="ExternalInput").ap()
    C2 = nc.dram_tensor("C2", [128, 135], dt.float32r, kind="ExternalInput").ap()
    C3 = nc.dram_tensor("C3", [128, 260], dt.float32r, kind="ExternalInput").ap()
    Wp = nc.dram_tensor("Wp", [3, 12], dt.float32r, kind="ExternalInput").ap()
    Hp = nc.dram_tensor("Hp", [68, 256], dt.float32r, kind="ExternalInput").ap()
    hw2 = nc.dram_tensor("hw2", [256, 64], dt.float32r, kind="ExternalInput").ap()
    BD = nc.dram_tensor("BD", [68, 3], dt.float32r, kind="ExternalInput").ap()
    BDT = nc.dram_tensor("BDT", [3, 68], dt.float32r, kind="ExternalInput").ap()
    BIAS = nc.dram_tensor("BIAS", [128, NBCOL], dt.float32, kind="ExternalInput").ap()
    ONES = nc.dram_tensor("ONES", [128, 128], dt.float32r, kind="ExternalInput").ap()
    probsT = nc.dram_tensor("probsT", [64, TOK], dt.float32, kind="ExternalOutput").ap()
    dbg = os.environ.get("KDEBUG", "0") == "1"
    if dbg:
        dbg_h0 = nc.dram_tensor("dbg_h0", [128, CHUNK], dt.float32, kind="ExternalOutput").ap()
        dbg_aa = nc.dram_tensor("dbg_aa", [1, CHUNK], dt.float32, kind="ExternalOutput").ap()
        dbg_u1 = nc.dram_tensor("dbg_u1", [128, CHUNK], dt.float32, kind="ExternalOutput").ap()
        dbg_er = nc.dram_tensor("dbg_er", [68, CHUNK], dt.float32, kind="ExternalOutput").ap()
        dbg_g = nc.dram_tensor("dbg_g", [128, CHUNK], dt.float32, kind="ExternalOutput").ap()
        dbg_E = nc.dram_tensor("dbg_E", [64, CHUNK], dt.float32, kind="ExternalOutput").ap()

    with tile.TileContext(nc) as tc, nc.allow_low_precision(reason="f32r tiles are 4-byte"):
        with tc.tile_pool(name="weights", bufs=1) as wp, \
             tc.tile_pool(name="x", bufs=2) as xp, \
             tc.tile_pool(name="inter", bufs=2) as ip, \
             tc.tile_pool(name="psB", bufs=4, space="PSUM") as psB, \
             tc.tile_pool(name="psM", bufs=3, space="PSUM") as psM, \
             tc.tile_pool(name="psC", bufs=1, space="PSUM") as psC:

            # ---- persistent weights ----
            pw1_t = []
            for k in range(16):
                t = wp.tile([128, 512], dt.bfloat16, name=f"pw1_{k}")
                nc.sync.dma_start(t[:], pw1[k * 128:(k + 1) * 128, :])
                pw1_t.append(t)
            pw2_t = []
            for k in range(4):
                t = wp.tile([128, 256], dt.float32r, name=f"pw2_{k}")
                nc.sync.dma_start(t[:], pw2[k * 128:(k + 1) * 128, :])
                pw2_t.append(t)
            lnA_t = []
            for k in range(2):
                t = wp.tile([128, 131], dt.float32r, name=f"lnA_{k}")
                nc.sync.dma_start(t[:], lnA[k * 128:(k + 1) * 128, :])
                lnA_t.append(t)
            crow_t = wp.tile([1, 131], dt.float32r, name="crow")
            nc.sync.dma_start(crow_t[:], CROW[:, :])
            c1_t = wp.tile([128, 135], dt.float32r, name="c1")
            nc.sync.dma_start(c1_t[:], C1[:, :])
            c2_t = wp.tile([128, 135], dt.float32r, name="c2")
            nc.sync.dma_start(c2_t[:], C2[:, :])
            c3_t = wp.tile([128, 260], dt.float32r, name="c3")
            nc.sync.dma_start(c3_t[:], C3[:, :])
            wp_t = wp.tile([3, 12], dt.float32r, name="wp_t")
            nc.sync.dma_start(wp_t[:], Wp[:, :])
            hp_t = wp.tile([68, 256], dt.float32r, name="hp_t")
            nc.sync.dma_start(hp_t[:], Hp[:, :])
            hw2_t = []
            for k in range(2):
                t = wp.tile([128, 64], dt.float32r, name=f"hw2_{k}")
                nc.sync.dma_start(t[:], hw2[k * 128:(k + 1) * 128, :])
                hw2_t.append(t)
            bd_t = wp.tile([68, 3], dt.float32r, name="bd_t")
            nc.sync.dma_start(bd_t[:], BD[:, :])
            bdt_t = wp.tile([3, 68], dt.float32r, name="bdt_t")
            nc.sync.dma_start(bdt_t[:], BDT[:, :])
            bias_t = wp.tile([128, NBCOL], dt.float32, name="bias_t")
            nc.sync.dma_start(bias_t[:], BIAS[:, :])
            ones_t = wp.tile([128, 128], dt.float32r, name="ones_t")
            nc.sync.dma_start(ones_t[:], ONES[:, :])

            def bcol(c, rows=128):
                return bias_t[0:rows, c:c + 1]

            # ---- per-chunk pipeline ----
            for ch in range(NCH):
                t0 = ch * CHUNK
                xt = []
                for k in range(16):
                    t = xp.tile([128, CHUNK], dt.bfloat16, tag=f"xt{k}", name=f"xt{k}_{ch}")
                    nc.sync.dma_start(t[:], xT[k * 128:(k + 1) * 128, t0:t0 + CHUNK])
                    xt.append(t)

                # input_proj layer 1: h1 = gelu(pw1.T @ xT + pb1)
                h1 = []
                for mo in range(4):
                    ps = psB.tile([128, CHUNK], dt.float32, tag="psb", name=f"ps_h1_{ch}_{mo}")
                    for k in range(16):
                        nc.tensor.matmul(ps[:], pw1_t[k][:, mo * 128:(mo + 1) * 128],
                                         xt[k][:], start=(k == 0), stop=(k == 15))
                    t = ip.tile([128, CHUNK], dt.float32r, tag=f"h1_{mo}", name=f"h1_{ch}_{mo}")
                    nc.scalar.activation(t[:], ps[:], AF.Gelu, bias=bcol(BCOL_PB1 + mo))
                    h1.append(t)

                # input_proj layer 2: h0 = pw2.T @ h1 + pb2
                h0 = []
                for mo in range(2):
                    ps = psB.tile([128, CHUNK], dt.float32, tag="psb", name=f"ps_h0_{ch}_{mo}")
                    for k in range(4):
                        nc.tensor.matmul(ps[:], pw2_t[k][:, mo * 128:(mo + 1) * 128],
                                         h1[k][:], start=(k == 0), stop=(k == 3))
                    t = ip.tile([128, CHUNK], dt.float32r, tag=f"h0_{mo}", name=f"h0_{ch}_{mo}")
                    nc.vector.tensor_scalar_add(t[:], ps[:], bcol(BCOL_PB2 + mo))
                    h0.append(t)

                # LayerNorm stats: sum and sum-of-squares over 256 features
                hsq = []
                for mo in range(2):
                    t = ip.tile([128, CHUNK], dt.float32r, tag=f"hsq_{mo}", name=f"hsq_{ch}_{mo}")
                    nc.vector.tensor_mul(t[:], h0[mo][:], h0[mo][:])
                    hsq.append(t)
                st0 = psM.tile([1, CHUNK], dt.float32, tag="psm", name=f"ps_st0_{ch}")
                for mo in range(2):
                    nc.tensor.matmul(st0[:], ones_t[:, 0:1], h0[mo][:],
                                     start=(mo == 0), stop=(mo == 1))
                st1 = psM.tile([1, CHUNK], dt.float32, tag="psm", name=f"ps_st1_{ch}")
                for mo in range(2):
                    nc.tensor.matmul(st1[:], ones_t[:, 0:1], hsq[mo][:],
                                     start=(mo == 0), stop=(mo == 1))
                # mu = st0/256 ; var = (st1 - st0^2/256)/256 ; aa = rstd ; bb = -mu*rstd
                tm = ip.tile([1, CHUNK], dt.float32, tag="scr", name=f"tm_{ch}")
                nc.scalar.activation(tm[:], st0[:], AF.Square)
                vv = ip.tile([1, CHUNK], dt.float32, tag="scr", name=f"vv_{ch}")
                nc.vector.scalar_tensor_tensor(vv[:], tm[:], -1.0 / 256.0, st1[:],
                                               op0=ALU.mult, op1=ALU.add)
                aa = ip.tile([1, CHUNK], dt.float32r, tag="aa", name=f"aa_{ch}")
                nc.scalar.activation(aa[:], vv[:], AF.Abs_reciprocal_sqrt,
                                     scale=1.0 / 256.0,
                                     bias=bias_t[0:1, BCOL_EPS:BCOL_EPS + 1])
                bb = ip.tile([1, CHUNK], dt.float32r, tag="bb", name=f"bb_{ch}")
                nc.vector.scalar_tensor_tensor(bb[:], st0[:], -1.0 / 256.0, aa[:],
                                               op0=ALU.mult, op1=ALU.mult)
                abc = psC.tile([128, CHUNK], dt.float32, tag="psc", name=f"ps_ab_{ch}")
                nc.tensor.matmul(abc[:], ones_t[0:1, 0:128], aa[:], start=True, stop=True)
                h0s = []
                for mo in range(2):
                    t = ip.tile([128, CHUNK], dt.float32r, tag=f"h0s_{mo}", name=f"h0s_{ch}_{mo}")
                    nc.vector.tensor_mul(t[:], h0[mo][:], abc[:])
                    h0s.append(t)

                # level 1 pre/pos:  hhat @ [A1|P1]  (+ rank-1 bb term)
                ps = psB.tile([128, CHUNK], dt.float32, tag="psb", name=f"ps_pre1_{ch}")
                for k in range(2):
                    nc.tensor.matmul(ps[:], lnA_t[k][:, 0:128], h0s[k][:],
                                     start=(k == 0), stop=False)
                nc.tensor.matmul(ps[:], crow_t[0:1, 0:128], bb[:], start=False, stop=True)
                u1 = ip.tile([128, CHUNK], dt.float32r, tag="u1", name=f"u1_{ch}")
                nc.scalar.activation(u1[:], ps[:], AF.Gelu, bias=bcol(BCOL_A1))

                pp = psM.tile([3, CHUNK], dt.float32, tag="psm", name=f"ps_pos1_{ch}")
                for k in range(2):
                    nc.tensor.matmul(pp[:], lnA_t[k][:, 128:131], h0s[k][:],
                                     start=(k == 0), stop=False)
                nc.tensor.matmul(pp[:], crow_t[0:1, 128:131], bb[:], start=False, stop=True)
                pos1 = ip.tile([3, CHUNK], dt.float32r, tag="pos", name=f"pos1_{ch}")
                nc.vector.tensor_scalar_add(pos1[:], pp[:], bcol(BCOL_P1, 3))

                erows = ip.tile([68, CHUNK], dt.float32r, tag="erows", name=f"erows_{ch}")

                # gelu chain: u2, u3 (keeps ACT on the gelu table back-to-back)
                ps = psB.tile([128, CHUNK], dt.float32, tag="psb", name=f"ps_pre2_{ch}")
                nc.tensor.matmul(ps[:], c1_t[:, 0:128], u1[:], start=True, stop=True)
                u2 = ip.tile([128, CHUNK], dt.float32r, tag="u2", name=f"u2_{ch}")
                nc.scalar.activation(u2[:], ps[:], AF.Gelu, bias=bcol(BCOL_C1A))
                ps = psB.tile([128, CHUNK], dt.float32, tag="psb", name=f"ps_pre3_{ch}")
                nc.tensor.matmul(ps[:], c2_t[:, 0:128], u2[:], start=True, stop=True)
                u3 = ip.tile([128, CHUNK], dt.float32r, tag="u3", name=f"u3_{ch}")
                nc.scalar.activation(u3[:], ps[:], AF.Gelu, bias=bcol(BCOL_C2A))

                # level logits psums + next-pos copies (DVE); exps deferred
                us = {1: u1, 2: u2, 3: u3}
                cts = {1: c1_t, 2: c2_t, 3: c3_t}
                pos = {1: pos1}
                pls = {}
                for lv in (1, 2, 3):
                    off = 128 if lv < 3 else 256
                    ct, u = cts[lv], us[lv]
                    pl = psM.tile([4, CHUNK], dt.float32, tag="psm", name=f"ps_bl{lv}_{ch}")
                    nc.tensor.matmul(pl[:], ct[:, off:off + 4], u[:], start=True, stop=False)
                    nc.tensor.matmul(pl[:], wp_t[:, 4 * (lv - 1):4 * lv], pos[lv][:],
                                     start=False, stop=True)
                    pls[lv] = pl
                    if lv < 3:
                        pw = psM.tile([3, CHUNK], dt.float32, tag="psm", name=f"ps_w3{lv}_{ch}")
                        nc.tensor.matmul(pw[:], ct[:, off + 4:off + 7], u[:],
                                         start=True, stop=True)
                        pn = ip.tile([3, CHUNK], dt.float32r, tag="pos", name=f"pos{lv + 1}_{ch}")
                        nc.vector.tensor_scalar_add(
                            pn[:], pw[:], bcol(BCOL_C1W if lv == 1 else BCOL_C2W, 3))
                        pos[lv + 1] = pn
                for lv, bcl in ((1, BCOL_B1), (2, BCOL_B2), (3, BCOL_B3)):
                    nc.scalar.activation(erows[32 * (lv - 1):32 * (lv - 1) + 4, :],
                                         pls[lv][:], AF.Exp, bias=bcol(bcl, 4))

                # level softmax: S = BD.T @ erows ; prows = erows * bcast(1/S)
                sl = psM.tile([3, CHUNK], dt.float32, tag="psm", name=f"ps_S_{ch}")
                nc.tensor.matmul(sl[:], bd_t[:], erows[:], start=True, stop=True)
                lrec = ip.tile([3, CHUNK], dt.float32r, tag="lrec", name=f"lrec_{ch}")
                nc.vector.reciprocal(lrec[:], sl[:])
                lbc = psC.tile([68, CHUNK], dt.float32, tag="psc", name=f"ps_lbc_{ch}")
                nc.tensor.matmul(lbc[:], bdt_t[:], lrec[:], start=True, stop=True)
                prows = ip.tile([68, CHUNK], dt.float32r, tag="prows", name=f"prows_{ch}")
                nc.vector.tensor_mul(prows[:], erows[:], lbc[:])

                # head: g = gelu(C3a.T @ u3 + Hp.T @ prows + bh)
                g = []
                for mo in range(2):
                    ps = psB.tile([128, CHUNK], dt.float32, tag="psb", name=f"ps_hd_{ch}_{mo}")
                    nc.tensor.matmul(ps[:], c3_t[:, mo * 128:(mo + 1) * 128], u3[:],
                                     start=True, stop=False)
                    nc.tensor.matmul(ps[:], hp_t[:, mo * 128:(mo + 1) * 128], prows[:],
                                     start=False, stop=True)
                    t = ip.tile([128, CHUNK], dt.float32r, tag=f"g_{mo}", name=f"g_{ch}_{mo}")
                    nc.scalar.activation(t[:], ps[:], AF.Gelu, bias=bcol(BCOL_BH + mo))
                    g.append(t)

                # expert logits -> DRAM (softmax + argmax on host)
                pe = psM.tile([64, CHUNK], dt.float32, tag="psm", name=f"ps_E_{ch}")
                for k in range(2):
                    nc.tensor.matmul(pe[:], hw2_t[k][:], g[k][:],
                                     start=(k == 0), stop=(k == 1))
                pout = ip.tile([64, CHUNK], dt.float32, tag="pout", name=f"pout_{ch}")
                nc.scalar.activation(pout[:], pe[:], AF.Identity, bias=bcol(BCOL_HB2, 64))
                nc.sync.dma_start(probsT[:, t0:t0 + CHUNK], pout[:])
                if dbg and ch == 0:
                    nc.sync.dma_start(dbg_h0[:, :], h0[0][:].bitcast(dt.float32))
                    nc.sync.dma_start(dbg_aa[:, :], aa[:].bitcast(dt.float32))
                    nc.sync.dma_start(dbg_u1[:, :], u1[:].bitcast(dt.float32))
                    nc.sync.dma_start(dbg_er[:, :], erows[:].bitcast(dt.float32))
                    nc.sync.dma_start(dbg_g[:, :], g[0][:].bitcast(dt.float32))
                    eL = ip.tile([64, CHUNK], dt.float32, tag="eL", name="eL_dbg")
                    nc.scalar.activation(eL[:], pe[:], AF.Identity)
                    nc.sync.dma_start(dbg_E[:, :], eL[:])

    nc.compile()
    return nc


def _prepare_in_maps(inputs):
    f = _fold(inputs)
    x = np.asarray(inputs["x"], np.float32)
    pw1_bf = np.ascontiguousarray(np.asarray(inputs["pw1"], np.float32)
                                  .astype(ml_dtypes.bfloat16))
    xbf = x.astype(ml_dtypes.bfloat16)
    shared = {
        "pw1": pw1_bf, "pw2": f["pw2"], "lnA": f["lnA"], "CROW": f["CROW"],
        "C1": f["C1"], "C2": f["C2"], "C3": f["C3"], "Wp": f["Wp"],
        "Hp": f["Hp"], "hw2": f["hw2"], "BD": f["BD"], "BDT": f["BDT"],
        "BIAS": f["BIAS"], "ONES": np.ones((128, 128), np.float32),
    }
    in_maps = []
    for c in range(N_CORES):
        xTc = np.ascontiguousarray(xbf[c * TOK:(c + 1) * TOK].T)
        in_maps.append({"xT": xTc, **shared})
    return in_maps


def kernel(**inputs):
    if "nc" not in _CACHE:
        _CACHE["nc"] = _build()
    nc = _CACHE["nc"]
    in_maps = _prepare_in_maps(inputs)
    res = run_bass_kernel_spmd(nc, in_maps, core_ids=list(range(N_CORES)))
    logitsT = np.concatenate([r["probsT"] for r in res.results], axis=1)
    L = logitsT.T.astype(np.float64)
    e = np.exp(L - L.max(-1, keepdims=True))
    p = e / e.sum(-1, keepdims=True)
    probs = p.astype(np.float32)
    ids = np.argmax(p, axis=-1).astype(np.int32)
    return probs, ids
